# revision 12
# baseline (speedup 1.0000x reference)
import os, sys

sys.path.insert(0, "/opt/trn_rl_repo")
ABL = int(os.environ.get("KABL", "0"))

import numpy as np
import ml_dtypes

import concourse.bass as bass
import concourse.bacc as bacc
import concourse.mybir as mybir
import concourse.tile as tile
from concourse.bass_utils import run_bass_kernel_spmd

BF = ml_dtypes.bfloat16
F32 = np.float32

N, G, S, V, R, L = 4096, 128, 256, 64, 32, 5
KA, NB = 16, 5
E_L, E_G = 49152, 126976
E = E_L + E_G
CUTOFF = 7.5
CORES = 8
NS = N // CORES          # 512 nodes per core
NW = NS // 128           # 4 dst windows per core
EG_SH = E_G // CORES     # 15872 bond edges per core
NTAB = N + 128           # gather tables padded with zero rows
BCH = 16                 # bond-head gather chunk (tiles)

dt = mybir.dt
AF = mybir.ActivationFunctionType
OP = mybir.AluOpType


def _wrap_idx(a):
    """int array [n] (n%16==0) -> wrapped int16 [128, n//16]."""
    n = a.shape[0]
    w = a.reshape(n // 16, 16).T.astype(np.int16)
    return np.tile(w, (8, 1))


def _build(TW):
    HT = TW // 4              # tiles per edge gather chunk (quarter window)
    CH = HT * 128             # idxs per edge gather
    NT = NW * TW              # edge tiles per core per layer
    EP = NT * 128             # padded edge slots per core
    NBT = EG_SH // 128        # bond tiles (124)

    nc = bacc.Bacc("TRN2", target_bir_lowering=False, debug=False,
                   num_devices=CORES)

    def din(name, shape, d=dt.bfloat16):
        return nc.dram_tensor(name, shape, d, kind="ExternalInput")

    i_eidx_src = din("eidx_src", [128, EP // 16], dt.int16)
    i_eidx_dst = din("eidx_dst", [128, EP // 16], dt.int16)
    i_own = din("own_idx", [128, NS // 16], dt.int16)
    i_bi = din("bond_i", [128, EG_SH // 16], dt.int16)
    i_bj = din("bond_j", [128, EG_SH // 16], dt.int16)
    i_rbfT = din("rbfT", [32, NT, 128])
    i_escal = din("escal", [NT, 128, 8], dt.float32)
    i_xT = din("xT_own", [KA, NS])
    i_ta2T = din("ta2T_own", [128, 2, NS], dt.float32)
    i_pc = din("pc_all", [N // 128, 128, 3], dt.float32)
    i_Wes = din("Wes", [L, 2, 128, S])
    i_Wed = din("Wed", [L, 2, 128, S])
    i_Wer = din("Wer", [L, 32, S])
    i_be = din("be", [L, 1, S])
    i_Ws = din("Ws", [L, 2, 128, S])
    i_bsT = din("bsT", [L, 128, 2], dt.float32)
    i_Wv = din("Wv", [L, 2, 128, V])
    i_Wvv = din("Wvv", [L, V, V])
    i_lgT = din("lgT", [L, 128, 2], dt.float32)
    i_lbT = din("lbT", [L, 128, 2], dt.float32)
    i_atW = din("atW", [KA, S])
    i_atW2 = din("atW2", [2, 128, S])
    i_atbT = din("atbT", [128, 2], dt.float32)
    i_shW = din("shW", [2, 128, S])
    i_shbT = din("shbT", [128, 2], dt.float32)
    i_b0W = din("b0W", [2, 128, S])
    i_wdB = din("wdB", [128, S], dt.float32)
    i_b0bB = din("b0bB", [128, S], dt.float32)
    i_b1W = din("b1W", [2, 128, NB])
    i_b1bB = din("b1bB", [128, NB], dt.float32)
    i_coW = din("coW", [V, 1])
    i_at2W = din("at2W", [2, 128, KA])
    i_at2bB = din("at2bB", [128, KA], dt.float32)
    i_B1 = din("B1", [32, 128, 128])
    i_B1T = din("B1T", [32, 128, 128])
    i_invc = din("invc", [128, 1], dt.float32)
    i_ident = din("ident", [128, 128])
    i_identf = din("identf", [128, 128], dt.float32)
    i_iota = din("iota", [128, 128], dt.float32)
    i_ones_r = din("ones_r", [1, 128])
    i_ones_rf = din("ones_rf", [1, 128], dt.float32)
    i_ones_cf = din("ones_cf", [128, 1], dt.float32)
    i_ones2 = din("ones2", [128, 2])

    o_coords = nc.dram_tensor("coords_sh", [NS, 3], dt.float32, kind="ExternalOutput")
    o_atoms = nc.dram_tensor("atoms_sh", [NS, KA], dt.float32, kind="ExternalOutput")
    o_bonds = nc.dram_tensor("bonds_sh", [EG_SH, NB], dt.float32, kind="ExternalOutput")

    with tile.TileContext(nc) as tc:
        with (
            tc.tile_pool(name="persist", bufs=1) as pp,
            tc.tile_pool(name="gath", bufs=2) as gp,
            tc.tile_pool(name="work", bufs=2) as wp,
            tc.tile_pool(name="small", bufs=2) as sp,
            tc.tile_pool(name="pagg", bufs=2, space="PSUM") as pagg,
            tc.tile_pool(name="pmm", bufs=2, space="PSUM") as pmm,
            tc.tile_pool(name="pt", bufs=2, space="PSUM") as pt,
            tc.tile_pool(name="pg", bufs=2, space="PSUM") as pg,
            tc.tile_pool(name="dram", bufs=2, space="DRAM") as dp,
            tc.tile_pool(name="dram1", bufs=1, space="DRAM") as dp1,
        ):
            # persistent state
            sT_bf = pp.tile([128, 2, N], dt.bfloat16)
            sT_f = pp.tile([128, 2, NS], dt.float32)
            vT_bf = pp.tile([V, 3, N], dt.bfloat16)
            vT_f = pp.tile([V, 3, NS], dt.float32)
            s2T = pp.tile([128, 2, N], dt.bfloat16)
            cvr = pp.tile([128, N // 128, 3], dt.float32)
            cvb = pp.tile([128, N // 128, 3], dt.bfloat16)

            _ldc = [0]

            def ld(shape, src, d=dt.bfloat16):
                _ldc[0] += 1
                tl = pp.tile(shape, d, tag=f"ld{_ldc[0]}")
                nc.sync.dma_start(tl[:], src)
                return tl

            ident = ld([128, 128], i_ident[:])
            identf = ld([128, 128], i_identf[:], dt.float32)
            iota = ld([128, 128], i_iota[:], dt.float32)
            ones_r = ld([1, 128], i_ones_r[:])
            ones_rf = ld([1, 128], i_ones_rf[:], dt.float32)
            ones_cf = ld([128, 1], i_ones_cf[:], dt.float32)
            ones2 = ld([128, 2], i_ones2[:])
            invc = ld([128, 1], i_invc[:], dt.float32)
            eidx_src = ld([128, EP // 16], i_eidx_src[:], dt.int16)
            eidx_dst = ld([128, EP // 16], i_eidx_dst[:], dt.int16)
            own_idx = ld([128, NS // 16], i_own[:], dt.int16)
            bond_i = ld([128, EG_SH // 16], i_bi[:], dt.int16)
            bond_j = ld([128, EG_SH // 16], i_bj[:], dt.int16)
            Wes = ld([128, L, 2, S], i_Wes.ap().rearrange("l k p n -> p l k n"))
            Wed = ld([128, L, 2, S], i_Wed.ap().rearrange("l k p n -> p l k n"))
            Wer = ld([32, L, S], i_Wer.ap().rearrange("l p n -> p l n"))
            be = ld([1, L, S], i_be.ap().rearrange("l o n -> o l n"))
            Wsp = ld([128, L, 2, S], i_Ws.ap().rearrange("l k p n -> p l k n"))
            bsT = ld([128, L, 2], i_bsT.ap().rearrange("l p c -> p l c"), dt.float32)
            Wv = ld([128, L, 2, V], i_Wv.ap().rearrange("l k p n -> p l k n"))
            Wvv = ld([V, L, V], i_Wvv.ap().rearrange("l p n -> p l n"))
            lgT = ld([128, L, 2], i_lgT.ap().rearrange("l p c -> p l c"), dt.float32)
            lbT = ld([128, L, 2], i_lbT.ap().rearrange("l p c -> p l c"), dt.float32)
            atW = ld([KA, S], i_atW[:])
            atW2 = ld([128, 2, S], i_atW2.ap().rearrange("k p n -> p k n"))
            atbT = ld([128, 2], i_atbT[:], dt.float32)
            shW = ld([128, 2, S], i_shW.ap().rearrange("k p n -> p k n"))
            shbT = ld([128, 2], i_shbT[:], dt.float32)
            b0W = ld([128, 2, S], i_b0W.ap().rearrange("k p n -> p k n"))
            wdB = ld([128, S], i_wdB[:], dt.float32)
            b0bB = ld([128, S], i_b0bB[:], dt.float32)
            b1W = ld([128, 2, NB], i_b1W.ap().rearrange("k p n -> p k n"))
            b1bB = ld([128, NB], i_b1bB[:], dt.float32)
            coW = ld([V, 1], i_coW[:])
            at2W = ld([128, 2, KA], i_at2W.ap().rearrange("k p n -> p k n"))
            at2bB = ld([128, KA], i_at2bB[:], dt.float32)

            # DRAM gather tables (zero-init incl. pad rows)
            tab_src = dp1.tile([NTAB, S], dt.bfloat16)
            tab_dst = dp1.tile([NTAB, S], dt.bfloat16)
            tab_vw = dp1.tile([NTAB, S], dt.bfloat16)
            tab_p2 = dp1.tile([NTAB, S], dt.bfloat16)
            tab_cof = dp1.tile([NTAB, V], dt.float32)
            zt = wp.tile([128, S], dt.bfloat16, tag="tabw")
            nc.vector.memset(zt[:], 0.0)
            ztf = wp.tile([128, 192], dt.float32, tag="aggv")
            nc.vector.memset(ztf[:], 0.0)
            for b in range(NTAB // 128):
                rsl = slice(b * 128, (b + 1) * 128)
                nc.sync.dma_start(tab_src[rsl, :], zt[:])
                nc.sync.dma_start(tab_dst[rsl, :], zt[:])
                nc.sync.dma_start(tab_vw[rsl, :], zt[:])
                nc.sync.dma_start(tab_p2[rsl, :], zt[:])
                nc.sync.dma_start(tab_cof[rsl, :], ztf[:, 0:V])

            nc.vector.memset(vT_bf[:], 0.0)
            nc.vector.memset(vT_f[:], 0.0)

            grp = [list(range(CORES))]

            def allgather_state():
                sb = wp.tile([128, 2, NS], dt.bfloat16, tag="sb_bnc")
                nc.vector.tensor_copy(sb[:], sT_f[:])
                b_in = dp.tile([128, 2, NS], dt.bfloat16, tag="bin_s")
                nc.sync.dma_start(b_in[:], sb[:])
                b_out = dp.tile([CORES, 128, 2, NS], dt.bfloat16, tag="bout_s")
                nc.gpsimd.collective_compute(
                    "AllGather", OP.bypass, replica_groups=grp,
                    ins=[b_in[:].opt()], outs=[b_out[:].opt()])
                nc.sync.dma_start(
                    sT_bf[:].rearrange("p f (c n) -> p f c n", c=CORES),
                    b_out[:].rearrange("c p f n -> p f c n"))
                vb = wp.tile([V, 3, NS], dt.bfloat16, tag="vb_bnc")
                nc.vector.tensor_copy(vb[:], vT_f[:])
                v_in = dp.tile([V, 3, NS], dt.bfloat16, tag="bin_v")
                nc.sync.dma_start(v_in[:], vb[:])
                v_out = dp.tile([CORES, V, 3, NS], dt.bfloat16, tag="bout_v")
                nc.gpsimd.collective_compute(
                    "AllGather", OP.bypass, replica_groups=grp,
                    ins=[v_in[:].opt()], outs=[v_out[:].opt()])
                nc.sync.dma_start(
                    vT_bf[:].rearrange("k d (c n) -> k d c n", c=CORES),
                    v_out[:].rearrange("c k d n -> k d c n"))

            # initial embedding (own nodes)
            xT = sp.tile([KA, NS], dt.bfloat16, tag="xT")
            nc.sync.dma_start(xT[:], i_xT[:])
            h1 = wp.tile([128, 2, NS], dt.bfloat16, tag="z")
            for fc in range(2):
                ta2f = wp.tile([128, NS], dt.float32, tag="z")
                nc.sync.dma_start(ta2f[:], i_ta2T[:, fc, :])
                ph = pmm.tile([128, 512], dt.float32, tag="mm")
                nc.tensor.matmul(ph[:], atW[:, fc * 128:(fc + 1) * 128],
                                 xT[:], start=True, stop=True)
                nc.vector.tensor_add(ph[:], ph[:], ta2f[:])
                nc.vector.tensor_copy(h1[:, fc, :], ph[:])
            for fc in range(2):
                ph = pmm.tile([128, 512], dt.float32, tag="mm")
                for kc in range(2):
                    nc.tensor.matmul(ph[:], atW2[:, kc, fc * 128:(fc + 1) * 128],
                                     h1[:, kc, :], start=(kc == 0), stop=(kc == 1))
                nc.vector.tensor_scalar_add(sT_f[:, fc, :], ph[:], atbT[:, fc:fc + 1])
            allgather_state()

            # ---------------- layers ----------------
            for l in range(L):
                for nt in range(N // 128):
                    rsl = slice(nt * 128, (nt + 1) * 128)
                    ps = pmm.tile([128, S], dt.float32, tag="mm")
                    for kc in range(2):
                        nc.tensor.matmul(ps[:], sT_bf[:, kc, rsl], Wes[:, l, kc, :],
                                         start=(kc == 0), stop=(kc == 1))
                    pb_ = wp.tile([128, S], dt.bfloat16, tag="tabw")
                    nc.vector.tensor_copy(pb_[:], ps[:])
                    nc.sync.dma_start(tab_src[rsl, :], pb_[:])
                    pd = pmm.tile([128, S], dt.float32, tag="mm")
                    for kc in range(2):
                        nc.tensor.matmul(pd[:], sT_bf[:, kc, rsl], Wed[:, l, kc, :],
                                         start=(kc == 0), stop=False)
                    nc.tensor.matmul(pd[:], ones_r[:], be[:, l, :],
                                     start=False, stop=True)
                    pdb = wp.tile([128, S], dt.bfloat16, tag="tabw")
                    nc.vector.tensor_copy(pdb[:], pd[:])
                    nc.sync.dma_start(tab_dst[rsl, :], pdb[:])
                    pv = pmm.tile([128, S], dt.float32, tag="mm")
                    for dd in range(3):
                        nc.tensor.matmul(pv[:, dd * V:(dd + 1) * V],
                                         vT_bf[:, dd, rsl], Wvv[:, l, :],
                                         start=True, stop=True)
                    pvb = wp.tile([128, 192], dt.bfloat16, tag="tabwv")
                    nc.vector.tensor_copy(pvb[:], pv[:, 0:192])
                    nc.sync.dma_start(tab_vw[rsl, 0:192], pvb[:])

                for w in range(NW if ABL < 3 else 0):
                    if ABL < 23:
                        psA = pagg.tile([128, 450], dt.float32)
                    for h in range(4):
                        cbase = w * TW + h * HT
                        csl = slice(cbase * 8, (cbase + HT) * 8)
                        pa = gp.tile([128, HT, S], dt.bfloat16, tag="pa")
                        nc.gpsimd.dma_gather(pa[:], tab_src[:], eidx_src[:, csl],
                                             num_idxs=CH, num_idxs_reg=CH,
                                             elem_size=S)
                        pbg = gp.tile([128, HT, S], dt.bfloat16, tag="pb")
                        nc.gpsimd.dma_gather(pbg[:], tab_dst[:], eidx_dst[:, csl],
                                             num_idxs=CH, num_idxs_reg=CH,
                                             elem_size=S)
                        vwg = gp.tile([128, HT, S], dt.bfloat16, tag="vw")
                        nc.gpsimd.dma_gather(vwg[:], tab_vw[:], eidx_src[:, csl],
                                             num_idxs=CH, num_idxs_reg=CH,
                                             elem_size=S)
                        for tt in range(HT if ABL < 25 else 0):
                            gt = cbase + tt
                            rbft = sp.tile([32, 128], dt.bfloat16, tag="rbft")
                            nc.sync.dma_start(rbft[:], i_rbfT[:, gt, :])
                            esc = sp.tile([128, 8], dt.float32, tag="esc")
                            nc.sync.dma_start(esc[:], i_escal[gt])
                            pm = pmm.tile([128, S], dt.float32, tag="mm")
                            nc.tensor.matmul(pm[:], rbft[:], Wer[:, l, :],
                                             start=True, stop=True)
                            mm = wp.tile([128, S], dt.float32, tag="mm")
                            nc.vector.tensor_add(mm[:], pa[:, tt, :], pbg[:, tt, :])
                            nc.vector.tensor_add(mm[:], mm[:], pm[:])
                            msil = wp.tile([128, S], dt.float32, tag="msil")
                            nc.scalar.activation(msil[:], mm[:], AF.Silu)
                            pay = wp.tile([128, 448], dt.bfloat16, tag="pay")
                            nc.vector.tensor_scalar_mul(pay[:, 0:S], msil[:],
                                                        esc[:, 3:4])
                            if ABL >= 24:
                                continue
                            mT = wp.tile([128, 2, 128], dt.bfloat16, tag="mT")
                            for kc in range(2):
                                px = pt.tile([128, 128], dt.bfloat16, tag="tr")
                                nc.tensor.transpose(
                                    px[:], pay[:, kc * 128:(kc + 1) * 128], ident[:])
                                nc.vector.tensor_copy(mT[:, kc, :], px[:])
                            pgt = pg.tile([128, V], dt.float32, tag="gate")
                            for kc in range(2):
                                nc.tensor.matmul(pgt[:], mT[:, kc, :],
                                                 Wv[:, l, kc, :],
                                                 start=(kc == 0), stop=(kc == 1))
                            for dd in range(3):
                                nc.vector.tensor_scalar_mul(
                                    pay[:, S + V * dd:S + V * (dd + 1)],
                                    pgt[:], esc[:, dd:dd + 1])
                            nc.vector.tensor_add(pay[:, S:S + 192],
                                                 pay[:, S:S + 192],
                                                 vwg[:, tt, 0:192])
                            if ABL >= 23:
                                continue
                            oh = wp.tile([128, 128], dt.bfloat16, tag="oh")
                            nc.vector.tensor_scalar(oh[:], iota[:], esc[:, 5:6],
                                                    None, op0=OP.is_equal)
                            st = (h == 0 and tt == 0)
                            sp_ = (h == 3 and tt == HT - 1)
                            if ABL == 22:
                                st = sp_ = True
                            nc.tensor.matmul(psA[:, 0:448], oh[:], pay[:],
                                             start=st, stop=sp_)
                            nc.tensor.matmul(psA[:, 448:450], oh[:], ones2[:],
                                             start=st, stop=sp_)
                    if ABL >= 23:
                        continue
                    wsl = slice(w * 128, (w + 1) * 128)  # epilogue runs for ABL<=22
                    cntc = sp.tile([128, 1], dt.float32, tag="cnt")
                    nc.vector.tensor_scalar_max(cntc[:], psA[:, 448:449], 1.0)
                    inv = sp.tile([128, 1], dt.float32, tag="inv")
                    nc.vector.reciprocal(inv[:], cntc[:])
                    aggm = wp.tile([128, S], dt.bfloat16, tag="aggm")
                    nc.vector.tensor_scalar_mul(aggm[:], psA[:, 0:S], inv[:])
                    aggv = wp.tile([128, 192], dt.float32, tag="aggv")
                    nc.vector.tensor_scalar_mul(aggv[:], psA[:, S:S + 192], inv[:])
                    aT = wp.tile([128, 2, 128], dt.bfloat16, tag="mT")
                    for kc in range(2):
                        px = pt.tile([128, 128], dt.bfloat16, tag="tr")
                        nc.tensor.transpose(px[:], aggm[:, kc * 128:(kc + 1) * 128],
                                            ident[:])
                        nc.vector.tensor_copy(aT[:, kc, :], px[:])
                    for fc in range(2):
                        ph = pt.tile([128, 128], dt.float32, tag="tr")
                        for kc in range(2):
                            nc.tensor.matmul(ph[:],
                                             Wsp[:, l, kc, fc * 128:(fc + 1) * 128],
                                             aT[:, kc, :], start=(kc == 0),
                                             stop=(kc == 1))
                        dl = wp.tile([128, 128], dt.float32, tag="dl")
                        nc.scalar.activation(dl[:], ph[:], AF.Silu,
                                             bias=bsT[:, l, fc:fc + 1])
                        nc.vector.tensor_add(sT_f[:, fc, wsl], sT_f[:, fc, wsl],
                                             dl[:])
                    for dd in range(3):
                        px = pt.tile([128, 128], dt.float32, tag="tr")
                        nc.tensor.transpose(px[0:V, :], aggv[:, dd * V:(dd + 1) * V],
                                            identf[:])
                        nc.vector.tensor_add(vT_f[:, dd, wsl], vT_f[:, dd, wsl],
                                             px[0:V, :])

                # layernorm on own nodes
                pmu = pmm.tile([1, NS], dt.float32, tag="mm")
                for fc in range(2):
                    nc.tensor.matmul(pmu[:], ones_cf[:], sT_f[:, fc, :],
                                     start=(fc == 0), stop=(fc == 1))
                pms = pmm.tile([1, NS], dt.float32, tag="mm")
                for fc in range(2):
                    sq2 = wp.tile([128, NS], dt.float32, tag="z")
                    nc.vector.tensor_tensor(sq2[:], sT_f[:, fc, :], sT_f[:, fc, :],
                                            op=OP.mult)
                    nc.tensor.matmul(pms[:], ones_cf[:], sq2[:],
                                     start=(fc == 0), stop=(fc == 1))
                mur = sp.tile([1, NS], dt.float32, tag="lnr")
                nc.vector.tensor_scalar_mul(mur[:], pmu[:], 1.0 / S)
                varr = sp.tile([1, NS], dt.float32, tag="lnr")
                nc.vector.tensor_tensor(varr[:], mur[:], mur[:], op=OP.mult)
                pMU = pmm.tile([128, NS], dt.float32, tag="mm")
                nc.tensor.matmul(pMU[:], ones_rf[:], mur[:], start=True, stop=True)
                msr = sp.tile([1, NS], dt.float32, tag="lnr")
                nc.vector.tensor_scalar_mul(msr[:], pms[:], 1.0 / S)
                nc.vector.tensor_sub(varr[:], msr[:], varr[:])
                nc.vector.tensor_scalar_add(varr[:], varr[:], 1e-5)
                sd = sp.tile([1, NS], dt.float32, tag="lnr")
                nc.scalar.activation(sd[:], varr[:], AF.Sqrt)
                rs = sp.tile([1, NS], dt.float32, tag="lnr")
                nc.vector.reciprocal(rs[:], sd[:])
                pRS = pmm.tile([128, NS], dt.float32, tag="mm")
                nc.tensor.matmul(pRS[:], ones_rf[:], rs[:], start=True, stop=True)
                for fc in range(2):
                    z = wp.tile([128, NS], dt.float32, tag="z")
                    nc.vector.tensor_sub(z[:], sT_f[:, fc, :], pMU[:])
                    nc.vector.tensor_tensor(z[:], z[:], pRS[:], op=OP.mult)
                    nc.vector.tensor_scalar(sT_f[:, fc, :], z[:],
                                            lgT[:, l, fc:fc + 1],
                                            lbT[:, l, fc:fc + 1],
                                            op0=OP.mult, op1=OP.add)
                allgather_state()

            # ---------------- heads ----------------
            for fc in range(2 if ABL < 2 else 0):
                for cb in range(N // 512):
                    sl = slice(cb * 512, (cb + 1) * 512)
                    ph = pmm.tile([128, 512], dt.float32, tag="mm")
                    for kc in range(2):
                        nc.tensor.matmul(ph[:], shW[:, kc, fc * 128:(fc + 1) * 128],
                                         sT_bf[:, kc, sl], start=(kc == 0),
                                         stop=(kc == 1))
                    s2f = wp.tile([128, 512], dt.float32, tag="z")
                    nc.scalar.activation(s2f[:], ph[:], AF.Silu,
                                         bias=shbT[:, fc:fc + 1])
                    nc.vector.tensor_copy(s2T[:, fc, sl], s2f[:])
            # p2 table + s2 row table (reuse tab_src)
            for nt in range(N // 128 if ABL < 2 else 0):
                rsl = slice(nt * 128, (nt + 1) * 128)
                ps = pmm.tile([128, S], dt.float32, tag="mm")
                for kc in range(2):
                    nc.tensor.matmul(ps[:], s2T[:, kc, rsl], b0W[:, kc, :],
                                     start=(kc == 0), stop=(kc == 1))
                pb_ = wp.tile([128, S], dt.bfloat16, tag="tabw")
                nc.vector.tensor_copy(pb_[:], ps[:])
                nc.sync.dma_start(tab_p2[rsl, :], pb_[:])
                s2r = wp.tile([128, S], dt.bfloat16, tag="tabw")
                for kc in range(2):
                    px = pt.tile([128, 128], dt.bfloat16, tag="tr")
                    nc.tensor.transpose(px[:], s2T[:, kc, rsl], ident[:])
                    nc.vector.tensor_copy(s2r[:, kc * 128:(kc + 1) * 128], px[:])
                nc.sync.dma_start(tab_src[rsl, :], s2r[:])

            # coords: cv = v @ co_W for all nodes, then center per graph
            for cb in range(N // 128 if ABL < 2 else 0):
                rsl = slice(cb * 128, (cb + 1) * 128)
                pcv = pg.tile([128, V], dt.float32, tag="gate")
                for dd in range(3):
                    nc.tensor.matmul(pcv[:, dd:dd + 1], vT_bf[:, dd, rsl], coW[:],
                                     start=True, stop=True)
                nc.vector.tensor_copy(cvr[:, cb, :], pcv[:, 0:3])
                nc.vector.tensor_copy(cvb[:, cb, :], pcv[:, 0:3])
            HEADS = ABL < 2
            B1 = gp.tile([128, 32, 128], dt.bfloat16, tag="pa")
            nc.sync.dma_start(B1[:], i_B1.ap().rearrange("c p g -> p c g"))
            B1T = gp.tile([128, 32, 128], dt.bfloat16, tag="pb")
            nc.sync.dma_start(B1T[:], i_B1T.ap().rearrange("c p g -> p c g"))
            pgm = pg.tile([128, V], dt.float32, tag="gate")
            for cb in range(N // 128 if HEADS else 0):
                nc.tensor.matmul(pgm[:, 0:3], B1[:, cb, :], cvb[:, cb, :],
                                 start=(cb == 0), stop=(cb == N // 128 - 1))
            if not HEADS:
                nc.tensor.matmul(pgm[:, 0:3], B1[:, 0, :], cvb[:, 0, :],
                                 start=True, stop=True)
            gm = sp.tile([128, 3], dt.float32, tag="gm")
            nc.vector.tensor_scalar_mul(gm[:], pgm[:, 0:3], invc[:])
            gmb = sp.tile([128, 3], dt.bfloat16, tag="gmb")
            nc.vector.tensor_copy(gmb[:], gm[:])
            ctf = wp.tile([128, V], dt.float32, tag="ctf")
            nc.vector.memset(ctf[:], 0.0)
            for cb in range(N // 128 if HEADS else 0):
                rsl = slice(cb * 128, (cb + 1) * 128)
                pe = pg.tile([128, V], dt.float32, tag="gate")
                nc.tensor.matmul(pe[:, 0:3], B1T[:, cb, :], gmb[:],
                                 start=True, stop=True)
                pct = sp.tile([128, 3], dt.float32, tag="pct")
                nc.sync.dma_start(pct[:], i_pc[cb])
                crw = sp.tile([128, 3], dt.float32, tag="crw")
                nc.vector.tensor_sub(crw[:], cvr[:, cb, :], pe[:, 0:3])
                nc.vector.tensor_add(crw[:], crw[:], pct[:])
                ctf2 = wp.tile([128, V], dt.float32, tag="ctf")
                nc.vector.tensor_copy(ctf2[:], ctf[:])
                nc.vector.tensor_copy(ctf2[:, 0:3], crw[:])
                nc.sync.dma_start(tab_cof[rsl, :], ctf2[:])

            # own coords + atoms via gathers
            if not HEADS:
                nc.vector.memset(cvb[:], 0.0)
            ocg = gp.tile([128, NS // 128, V], dt.float32, tag="ci")
            nc.gpsimd.dma_gather(ocg[:], tab_cof[:], own_idx[:], num_idxs=NS,
                                 num_idxs_reg=NS, elem_size=V)
            for cb in range(NS // 128):
                nc.sync.dma_start(
                    o_coords.ap().rearrange("(a p) d -> p a d", p=128)[:, cb, :],
                    ocg[:, cb, 0:3])
            os2 = gp.tile([128, NS // 128, S], dt.bfloat16, tag="pa")
            nc.gpsimd.dma_gather(os2[:], tab_src[:], own_idx[:], num_idxs=NS,
                                 num_idxs_reg=NS, elem_size=S)
            for cb in range(NS // 128):
                s2To = wp.tile([128, 2, 128], dt.bfloat16, tag="mT")
                for kc in range(2):
                    px = pt.tile([128, 128], dt.bfloat16, tag="tr")
                    nc.tensor.transpose(px[:], os2[:, cb, kc * 128:(kc + 1) * 128],
                                        ident[:])
                    nc.vector.tensor_copy(s2To[:, kc, :], px[:])
                pat = pg.tile([128, V], dt.float32, tag="gate")
                for kc in range(2):
                    nc.tensor.matmul(pat[:, 0:KA], s2To[:, kc, :], at2W[:, kc, :],
                                     start=(kc == 0), stop=(kc == 1))
                arow = sp.tile([128, KA], dt.float32, tag="arow")
                nc.vector.tensor_add(arow[:], pat[:, 0:KA], at2bB[:])
                nc.sync.dma_start(
                    o_atoms.ap().rearrange("(a p) d -> p a d", p=128)[:, cb, :],
                    arow[:])

            # bonds head
            nbc = (NBT + BCH - 1) // BCH if ABL < 1 else 0
            for c in range(nbc):
                t0 = c * BCH
                ct = min(BCH, NBT - t0)
                cn = ct * 128
                csl = slice(t0 * 8, t0 * 8 + cn // 16)
                fi = gp.tile([128, BCH, S], dt.bfloat16, tag="pa")
                nc.gpsimd.dma_gather(fi[:, 0:ct, :], tab_p2[:], bond_i[:, csl],
                                     num_idxs=cn, num_idxs_reg=cn, elem_size=S)
                fj = gp.tile([128, BCH, S], dt.bfloat16, tag="pb")
                nc.gpsimd.dma_gather(fj[:, 0:ct, :], tab_p2[:], bond_j[:, csl],
                                     num_idxs=cn, num_idxs_reg=cn, elem_size=S)
                ci = gp.tile([128, BCH, V], dt.float32, tag="ci")
                nc.gpsimd.dma_gather(ci[:, 0:ct, :], tab_cof[:], bond_i[:, csl],
                                     num_idxs=cn, num_idxs_reg=cn, elem_size=V)
                cj = gp.tile([128, BCH, V], dt.float32, tag="cj")
                nc.gpsimd.dma_gather(cj[:, 0:ct, :], tab_cof[:], bond_j[:, csl],
                                     num_idxs=cn, num_idxs_reg=cn, elem_size=V)
                for tt in range(ct):
                    gt = t0 + tt
                    dsub = sp.tile([128, 3], dt.float32, tag="dsub")
                    nc.vector.tensor_sub(dsub[:], ci[:, tt, 0:3], cj[:, tt, 0:3])
                    dq = sp.tile([128, 3], dt.float32, tag="dq")
                    nc.vector.tensor_tensor(dq[:], dsub[:], dsub[:], op=OP.mult)
                    ds = sp.tile([128, 1], dt.float32, tag="ds")
                    nc.vector.tensor_reduce(ds[:], dq[:], axis=mybir.AxisListType.X,
                                            op=OP.add)
                    nc.vector.tensor_scalar_max(ds[:], ds[:], 1e-12)
                    dcol = sp.tile([128, 1], dt.float32, tag="dcol")
                    nc.scalar.activation(dcol[:], ds[:], AF.Sqrt)
                    e1 = wp.tile([128, S], dt.float32, tag="mm")
                    nc.vector.tensor_scalar_mul(e1[:], wdB[:], dcol[:])
                    nc.vector.tensor_add(e1[:], e1[:], b0bB[:])
                    nc.vector.tensor_add(e1[:], e1[:], fi[:, tt, :])
                    nc.vector.tensor_add(e1[:], e1[:], fj[:, tt, :])
                    es = wp.tile([128, S], dt.float32, tag="msil")
                    nc.scalar.activation(es[:], e1[:], AF.Silu)
                    esb = wp.tile([128, S], dt.bfloat16, tag="pay")
                    nc.vector.tensor_copy(esb[:], es[:])
                    eT = wp.tile([128, 2, 128], dt.bfloat16, tag="mT")
                    for kc in range(2):
                        px = pt.tile([128, 128], dt.bfloat16, tag="tr")
                        nc.tensor.transpose(px[:], esb[:, kc * 128:(kc + 1) * 128],
                                            ident[:])
                        nc.vector.tensor_copy(eT[:, kc, :], px[:])
                    pb5 = pg.tile([128, V], dt.float32, tag="gate")
                    for kc in range(2):
                        nc.tensor.matmul(pb5[:, 0:NB], eT[:, kc, :], b1W[:, kc, :],
                                         start=(kc == 0), stop=(kc == 1))
                    ob = sp.tile([128, NB], dt.float32, tag="ob")
                    nc.vector.tensor_add(ob[:], pb5[:, 0:NB], b1bB[:])
                    nc.sync.dma_start(
                        o_bonds.ap().rearrange("(a p) d -> p a d", p=128)[:, gt, :],
                        ob[:])

    nc.compile()
    return nc


_CACHE = {}


def kernel(x, t, pos, edge_index_local, edge_index_global, batch, params):
    x = np.asarray(x, F32)
    t = np.asarray(t, F32)
    pos = np.asarray(pos, F32)
    eil = np.asarray(edge_index_local).astype(np.int64)
    eig = np.asarray(edge_index_global).astype(np.int64)
    batch = np.asarray(batch).astype(np.int64)
    p = {k: np.asarray(v, F32) for k, v in params.items()}

    # ---- host prep: geometry ----
    cnt = np.bincount(batch, minlength=G).astype(F32)
    cnt1 = np.maximum(cnt, 1.0)
    gsum = np.zeros((G, 3), F32)
    np.add.at(gsum, batch, pos)
    pos_c = pos - (gsum / cnt1[:, None])[batch]
    gsum2 = np.zeros((G, 3), F32)
    np.add.at(gsum2, batch, pos_c)
    pc = pos_c - (gsum2 / cnt1[:, None])[batch]

    src = np.concatenate([eil[0], eig[0]])
    dst = np.concatenate([eil[1], eig[1]])
    rvec = pos_c[dst] - pos_c[src]
    d = np.sqrt(np.maximum((rvec * rvec).sum(-1), 1e-6))
    rn = rvec / d[:, None]
    dL = d[:E_L]
    env = np.concatenate([
        (0.5 * (np.cos(np.pi * dL / CUTOFF) + 1.0) * (dL < CUTOFF)).astype(F32),
        np.ones(E_G, F32)])
    centers = np.linspace(0.0, CUTOFF, R).astype(F32)
    rbf = np.exp(-(R / CUTOFF) * (d[:, None] - centers) ** 2).astype(F32)

    # ---- sort edges by dst, shard by dst range, window by 128 ----
    order = np.argsort(dst, kind="stable")
    dst_s = dst[order]
    starts = np.searchsorted(dst_s, np.arange(0, N + 1, 128))
    wcnt = np.diff(starts)          # edges per (core,window) flat [32]
    TW = int(np.ceil(wcnt.max() / 128.0))
    TW = ((TW + 3) // 4) * 4
    NT = NW * TW
    EP = NT * 128

    key = TW
    if key not in _CACHE:
        _CACHE[key] = _build(TW)
    nc = _CACHE[key]

    def bc(a):
        return np.ascontiguousarray(a).astype(BF)

    # per-core padded edge arrays
    in_maps = []
    eye = np.eye(128, dtype=F32)
    iota = np.tile(np.arange(128, dtype=F32)[None, :], (128, 1))
    common = dict(
        Wes=bc(p["We"][:, :S].reshape(L, 2, 128, S)),
        Wed=bc(p["We"][:, S:2 * S].reshape(L, 2, 128, S)),
        Wer=bc(p["We"][:, 2 * S:]),
        be=bc(p["be"].reshape(L, 1, S)),
        Ws=bc(p["Ws"].reshape(L, 2, 128, S)),
        bsT=np.ascontiguousarray(
            p["bs"].reshape(L, 2, 128).transpose(0, 2, 1)).astype(F32),
        Wv=bc(p["Wv"].reshape(L, 2, 128, V)),
        Wvv=bc(p["Wvv"]),
        lgT=np.ascontiguousarray(
            p["ln_g"].reshape(L, 2, 128).transpose(0, 2, 1)).astype(F32),
        lbT=np.ascontiguousarray(
            p["ln_b"].reshape(L, 2, 128).transpose(0, 2, 1)).astype(F32),
        atW=bc(p["atom_W"]),
        atW2=bc(p["at_W"].reshape(2, 128, S)),
        atbT=np.ascontiguousarray(p["at_b"].reshape(2, 128).T).astype(F32),
        shW=bc(p["sh_W"].reshape(2, 128, S)),
        shbT=np.ascontiguousarray(p["sh_b"].reshape(2, 128).T).astype(F32),
        b0W=bc(p["b0_W"][:S].reshape(2, 128, S)),
        wdB=np.tile(p["b0_W"][S][None, :], (128, 1)).astype(F32),
        b0bB=np.tile(p["b0_b"][None, :], (128, 1)).astype(F32),
        b1W=bc(p["b1_W"].reshape(2, 128, NB)),
        b1bB=np.tile(p["b1_b"][None, :], (128, 1)).astype(F32),
        coW=bc(p["co_W"]),
        at2W=bc(p["at2_W"].reshape(2, 128, KA)),
        at2bB=np.tile(p["at2_b"][None, :], (128, 1)).astype(F32),
        invc=(1.0 / cnt1).reshape(128, 1).astype(F32),
        ident=bc(eye),
        identf=eye.astype(F32),
        iota=iota.astype(F32),
        ones_r=bc(np.ones((1, 128), F32)),
        ones_rf=np.ones((1, 128), F32),
        ones_cf=np.ones((128, 1), F32),
        ones2=np.ones((128, 2), BF),
        pc_all=np.ascontiguousarray(pc.reshape(N // 128, 128, 3)).astype(F32),
    )
    B1 = np.zeros((N, G), F32)
    B1[np.arange(N), batch] = 1.0
    common["B1"] = bc(B1.reshape(32, 128, 128))
    common["B1T"] = bc(
        np.ascontiguousarray(B1.T).reshape(128, 32, 128).transpose(1, 0, 2))

    ta = (t @ p["time_W"] + p["time_b"]).astype(F32)
    ta2 = ta[batch] + p["atom_b"][None, :]

    for c in range(CORES):
        src_p = np.full(EP, N, np.int64)
        dst_p = np.full(EP, N, np.int64)
        esc = np.zeros((EP, 8), F32)
        esc[:, 5] = -1.0
        rbf_p = np.zeros((EP, R), F32)
        for w in range(NW):
            gwi = c * NW + w
            s0, s1 = starts[gwi], starts[gwi + 1]
            idxs = order[s0:s1]
            n = s1 - s0
            base = w * TW * 128
            src_p[base:base + n] = src[idxs]
            dst_p[base:base + n] = dst[idxs]
            esc[base:base + n, 0:3] = rn[idxs]
            esc[base:base + n, 3] = env[idxs]
            esc[base:base + n, 4] = 1.0
            esc[base:base + n, 5] = (dst[idxs] - (c * NS + w * 128)).astype(F32)
            rbf_p[base:base + n] = rbf[idxs]
        own = np.arange(c * NS, (c + 1) * NS)
        m = dict(common)
        m.update(
            eidx_src=_wrap_idx(src_p),
            eidx_dst=_wrap_idx(dst_p),
            own_idx=_wrap_idx(own),
            bond_i=_wrap_idx(eig[1][c * EG_SH:(c + 1) * EG_SH]),
            bond_j=_wrap_idx(eig[0][c * EG_SH:(c + 1) * EG_SH]),
            rbfT=bc(rbf_p.reshape(NT, 128, R).transpose(2, 0, 1)),
            escal=np.ascontiguousarray(esc.reshape(NT, 128, 8)),
            xT_own=bc(x[own].T),
            ta2T_own=np.ascontiguousarray(
                ta2[own].T.reshape(2, 128, NS).transpose(1, 0, 2)).astype(F32),
        )
        in_maps.append(m)

    res = run_bass_kernel_spmd(nc, in_maps, core_ids=list(range(CORES)),
                               trace=False)
    coords = np.concatenate([r["coords_sh"] for r in res.results], 0)
    atoms = np.concatenate([r["atoms_sh"] for r in res.results], 0)
    bonds = np.concatenate([r["bonds_sh"] for r in res.results], 0)
    return coords, atoms, bonds


# revision 13
# speedup vs baseline: 1.0439x; 1.0439x over previous
import os, sys

sys.path.insert(0, "/opt/trn_rl_repo")
ABL = int(os.environ.get("KABL", "0"))

import numpy as np
import ml_dtypes

import concourse.bass as bass
import concourse.bacc as bacc
import concourse.mybir as mybir
import concourse.tile as tile
from concourse.bass_utils import run_bass_kernel_spmd

BF = ml_dtypes.bfloat16
F32 = np.float32

N, G, S, V, R, L = 4096, 128, 256, 64, 32, 5
KA, NB = 16, 5
E_L, E_G = 49152, 126976
E = E_L + E_G
CUTOFF = 7.5
CORES = 8
NS = N // CORES          # 512 nodes per core
NW = NS // 128           # 4 dst windows per core
EG_SH = E_G // CORES     # 15872 bond edges per core
NTAB = N + 128           # gather tables padded with zero rows
BCH = 16                 # bond-head gather chunk (tiles)

dt = mybir.dt
AF = mybir.ActivationFunctionType
OP = mybir.AluOpType


def _wrap_idx(a):
    """int array [n] (n%16==0) -> wrapped int16 [128, n//16]."""
    n = a.shape[0]
    w = a.reshape(n // 16, 16).T.astype(np.int16)
    return np.tile(w, (8, 1))


def _build(TW):
    HT = TW // 4              # tiles per edge gather chunk (quarter window)
    CH = HT * 128             # idxs per edge gather
    NT = NW * TW              # edge tiles per core per layer
    EP = NT * 128             # padded edge slots per core
    NBT = EG_SH // 128        # bond tiles (124)

    nc = bacc.Bacc("TRN2", target_bir_lowering=False, debug=False,
                   num_devices=CORES)

    def din(name, shape, d=dt.bfloat16):
        return nc.dram_tensor(name, shape, d, kind="ExternalInput")

    i_eidx_src = din("eidx_src", [128, EP // 16], dt.int16)
    i_eidx_dst = din("eidx_dst", [128, EP // 16], dt.int16)
    i_own = din("own_idx", [128, NS // 16], dt.int16)
    i_bi = din("bond_i", [128, EG_SH // 16], dt.int16)
    i_bj = din("bond_j", [128, EG_SH // 16], dt.int16)
    i_rbfT = din("rbfT", [32, NT, 128])
    i_escal = din("escal", [NT, 128, 8], dt.float32)
    i_xT = din("xT_own", [KA, NS])
    i_ta2T = din("ta2T_own", [128, 2, NS], dt.float32)
    i_pc = din("pc_all", [N // 128, 128, 3], dt.float32)
    i_Wes = din("Wes", [L, 2, 128, S])
    i_Wed = din("Wed", [L, 2, 128, S])
    i_Wer = din("Wer", [L, 32, S])
    i_be = din("be", [L, 1, S])
    i_Ws = din("Ws", [L, 2, 128, S])
    i_bsT = din("bsT", [L, 128, 2], dt.float32)
    i_Wv = din("Wv", [L, 2, 128, V])
    i_Wvv = din("Wvv", [L, V, V])
    i_lgT = din("lgT", [L, 128, 2], dt.float32)
    i_lbT = din("lbT", [L, 128, 2], dt.float32)
    i_atW = din("atW", [KA, S])
    i_atW2 = din("atW2", [2, 128, S])
    i_atbT = din("atbT", [128, 2], dt.float32)
    i_shW = din("shW", [2, 128, S])
    i_shbT = din("shbT", [128, 2], dt.float32)
    i_b0W = din("b0W", [2, 128, S])
    i_wdB = din("wdB", [128, S], dt.float32)
    i_b0bB = din("b0bB", [128, S], dt.float32)
    i_b1W = din("b1W", [2, 128, NB])
    i_b1bB = din("b1bB", [128, NB], dt.float32)
    i_coW = din("coW", [V, 1])
    i_at2W = din("at2W", [2, 128, KA])
    i_at2bB = din("at2bB", [128, KA], dt.float32)
    i_B1 = din("B1", [32, 128, 128])
    i_B1T = din("B1T", [32, 128, 128])
    i_invc = din("invc", [128, 1], dt.float32)
    i_ident = din("ident", [128, 128])
    i_identf = din("identf", [128, 128], dt.float32)
    i_iota = din("iota", [128, 128], dt.float32)
    i_ones_r = din("ones_r", [1, 128])
    i_ones_rf = din("ones_rf", [1, 128], dt.float32)
    i_ones_cf = din("ones_cf", [128, 1], dt.float32)
    i_ones2 = din("ones2", [128, 2])

    o_coords = nc.dram_tensor("coords_sh", [NS, 3], dt.float32, kind="ExternalOutput")
    o_atoms = nc.dram_tensor("atoms_sh", [NS, KA], dt.float32, kind="ExternalOutput")
    o_bonds = nc.dram_tensor("bonds_sh", [EG_SH, NB], dt.float32, kind="ExternalOutput")

    with tile.TileContext(nc) as tc:
        with (
            tc.tile_pool(name="persist", bufs=1) as pp,
            tc.tile_pool(name="gath", bufs=2) as gp,
            tc.tile_pool(name="work", bufs=2) as wp,
            tc.tile_pool(name="small", bufs=2) as sp,
            tc.tile_pool(name="pagg", bufs=2, space="PSUM") as pagg,
            tc.tile_pool(name="pmm", bufs=2, space="PSUM") as pmm,
            tc.tile_pool(name="pt", bufs=2, space="PSUM") as pt,
            tc.tile_pool(name="pg", bufs=2, space="PSUM") as pg,
            tc.tile_pool(name="dram", bufs=2, space="DRAM") as dp,
            tc.tile_pool(name="dram1", bufs=1, space="DRAM") as dp1,
        ):
            # persistent state
            sT_bf = pp.tile([128, 2, N], dt.bfloat16)
            sT_f = pp.tile([128, 2, NS], dt.float32)
            vT_bf = pp.tile([V, 3, N], dt.bfloat16)
            vT_f = pp.tile([V, 3, NS], dt.float32)
            s2T = pp.tile([128, 2, N], dt.bfloat16)
            cvr = pp.tile([128, N // 128, 3], dt.float32)
            cvb = pp.tile([128, N // 128, 3], dt.bfloat16)

            _ldc = [0]

            def ld(shape, src, d=dt.bfloat16):
                _ldc[0] += 1
                tl = pp.tile(shape, d, tag=f"ld{_ldc[0]}")
                nc.sync.dma_start(tl[:], src)
                return tl

            ident = ld([128, 128], i_ident[:])
            identf = ld([128, 128], i_identf[:], dt.float32)
            iota = ld([128, 128], i_iota[:], dt.float32)
            ones_r = ld([1, 128], i_ones_r[:])
            ones_rf = ld([1, 128], i_ones_rf[:], dt.float32)
            ones_cf = ld([128, 1], i_ones_cf[:], dt.float32)
            ones2 = ld([128, 2], i_ones2[:])
            invc = ld([128, 1], i_invc[:], dt.float32)
            eidx_src = ld([128, EP // 16], i_eidx_src[:], dt.int16)
            eidx_dst = ld([128, EP // 16], i_eidx_dst[:], dt.int16)
            own_idx = ld([128, NS // 16], i_own[:], dt.int16)
            bond_i = ld([128, EG_SH // 16], i_bi[:], dt.int16)
            bond_j = ld([128, EG_SH // 16], i_bj[:], dt.int16)
            Wes = ld([128, L, 2, S], i_Wes.ap().rearrange("l k p n -> p l k n"))
            Wed = ld([128, L, 2, S], i_Wed.ap().rearrange("l k p n -> p l k n"))
            Wer = ld([32, L, S], i_Wer.ap().rearrange("l p n -> p l n"))
            be = ld([1, L, S], i_be.ap().rearrange("l o n -> o l n"))
            Wsp = ld([128, L, 2, S], i_Ws.ap().rearrange("l k p n -> p l k n"))
            bsT = ld([128, L, 2], i_bsT.ap().rearrange("l p c -> p l c"), dt.float32)
            Wv = ld([128, L, 2, V], i_Wv.ap().rearrange("l k p n -> p l k n"))
            Wvv = ld([V, L, V], i_Wvv.ap().rearrange("l p n -> p l n"))
            lgT = ld([128, L, 2], i_lgT.ap().rearrange("l p c -> p l c"), dt.float32)
            lbT = ld([128, L, 2], i_lbT.ap().rearrange("l p c -> p l c"), dt.float32)
            atW = ld([KA, S], i_atW[:])
            atW2 = ld([128, 2, S], i_atW2.ap().rearrange("k p n -> p k n"))
            atbT = ld([128, 2], i_atbT[:], dt.float32)
            shW = ld([128, 2, S], i_shW.ap().rearrange("k p n -> p k n"))
            shbT = ld([128, 2], i_shbT[:], dt.float32)
            b0W = ld([128, 2, S], i_b0W.ap().rearrange("k p n -> p k n"))
            wdB = ld([128, S], i_wdB[:], dt.float32)
            b0bB = ld([128, S], i_b0bB[:], dt.float32)
            b1W = ld([128, 2, NB], i_b1W.ap().rearrange("k p n -> p k n"))
            b1bB = ld([128, NB], i_b1bB[:], dt.float32)
            coW = ld([V, 1], i_coW[:])
            at2W = ld([128, 2, KA], i_at2W.ap().rearrange("k p n -> p k n"))
            at2bB = ld([128, KA], i_at2bB[:], dt.float32)

            # DRAM gather tables (zero-init incl. pad rows)
            tab_src = dp1.tile([NTAB, S], dt.bfloat16)
            tab_dst = dp1.tile([NTAB, S], dt.bfloat16)
            tab_vw = dp1.tile([NTAB, S], dt.bfloat16)
            tab_p2 = dp1.tile([NTAB, S], dt.bfloat16)
            tab_cof = dp1.tile([NTAB, V], dt.float32)
            zt = wp.tile([128, S], dt.bfloat16, tag="tabw")
            nc.vector.memset(zt[:], 0.0)
            ztf = wp.tile([128, 192], dt.float32, tag="aggv")
            nc.vector.memset(ztf[:], 0.0)
            for b in range(NTAB // 128):
                rsl = slice(b * 128, (b + 1) * 128)
                nc.sync.dma_start(tab_src[rsl, :], zt[:])
                nc.sync.dma_start(tab_dst[rsl, :], zt[:])
                nc.sync.dma_start(tab_vw[rsl, :], zt[:])
                nc.sync.dma_start(tab_p2[rsl, :], zt[:])
                nc.sync.dma_start(tab_cof[rsl, :], ztf[:, 0:V])

            nc.vector.memset(vT_bf[:], 0.0)
            nc.vector.memset(vT_f[:], 0.0)

            grp = [list(range(CORES))]

            def allgather_state():
                sb = wp.tile([128, 2, NS], dt.bfloat16, tag="sb_bnc")
                nc.vector.tensor_copy(sb[:], sT_f[:])
                b_in = dp.tile([128, 2, NS], dt.bfloat16, tag="bin_s")
                nc.sync.dma_start(b_in[:], sb[:])
                b_out = dp.tile([CORES, 128, 2, NS], dt.bfloat16, tag="bout_s")
                nc.gpsimd.collective_compute(
                    "AllGather", OP.bypass, replica_groups=grp,
                    ins=[b_in[:].opt()], outs=[b_out[:].opt()])
                nc.sync.dma_start(
                    sT_bf[:].rearrange("p f (c n) -> p f c n", c=CORES),
                    b_out[:].rearrange("c p f n -> p f c n"))
                vb = wp.tile([V, 3, NS], dt.bfloat16, tag="vb_bnc")
                nc.vector.tensor_copy(vb[:], vT_f[:])
                v_in = dp.tile([V, 3, NS], dt.bfloat16, tag="bin_v")
                nc.sync.dma_start(v_in[:], vb[:])
                v_out = dp.tile([CORES, V, 3, NS], dt.bfloat16, tag="bout_v")
                nc.gpsimd.collective_compute(
                    "AllGather", OP.bypass, replica_groups=grp,
                    ins=[v_in[:].opt()], outs=[v_out[:].opt()])
                nc.sync.dma_start(
                    vT_bf[:].rearrange("k d (c n) -> k d c n", c=CORES),
                    v_out[:].rearrange("c k d n -> k d c n"))

            # initial embedding (own nodes)
            xT = sp.tile([KA, NS], dt.bfloat16, tag="xT")
            nc.sync.dma_start(xT[:], i_xT[:])
            h1 = wp.tile([128, 2, NS], dt.bfloat16, tag="z")
            for fc in range(2):
                ta2f = wp.tile([128, NS], dt.float32, tag="z")
                nc.sync.dma_start(ta2f[:], i_ta2T[:, fc, :])
                ph = pmm.tile([128, 512], dt.float32, tag="mm")
                nc.tensor.matmul(ph[:], atW[:, fc * 128:(fc + 1) * 128],
                                 xT[:], start=True, stop=True)
                nc.vector.tensor_add(ph[:], ph[:], ta2f[:])
                nc.vector.tensor_copy(h1[:, fc, :], ph[:])
            for fc in range(2):
                ph = pmm.tile([128, 512], dt.float32, tag="mm")
                for kc in range(2):
                    nc.tensor.matmul(ph[:], atW2[:, kc, fc * 128:(fc + 1) * 128],
                                     h1[:, kc, :], start=(kc == 0), stop=(kc == 1))
                nc.vector.tensor_scalar_add(sT_f[:, fc, :], ph[:], atbT[:, fc:fc + 1])
            allgather_state()

            # ---------------- layers ----------------
            for l in range(L):
                for nt in range(N // 128):
                    rsl = slice(nt * 128, (nt + 1) * 128)
                    ps = pmm.tile([128, S], dt.float32, tag="mm")
                    for kc in range(2):
                        nc.tensor.matmul(ps[:], sT_bf[:, kc, rsl], Wes[:, l, kc, :],
                                         start=(kc == 0), stop=(kc == 1))
                    pb_ = wp.tile([128, S], dt.bfloat16, tag="tabw")
                    nc.vector.tensor_copy(pb_[:], ps[:])
                    nc.sync.dma_start(tab_src[rsl, :], pb_[:])
                    pd = pmm.tile([128, S], dt.float32, tag="mm")
                    for kc in range(2):
                        nc.tensor.matmul(pd[:], sT_bf[:, kc, rsl], Wed[:, l, kc, :],
                                         start=(kc == 0), stop=False)
                    nc.tensor.matmul(pd[:], ones_r[:], be[:, l, :],
                                     start=False, stop=True)
                    pdb = wp.tile([128, S], dt.bfloat16, tag="tabw")
                    nc.vector.tensor_copy(pdb[:], pd[:])
                    nc.sync.dma_start(tab_dst[rsl, :], pdb[:])
                    pv = pmm.tile([128, S], dt.float32, tag="mm")
                    for dd in range(3):
                        nc.tensor.matmul(pv[:, dd * V:(dd + 1) * V],
                                         vT_bf[:, dd, rsl], Wvv[:, l, :],
                                         start=True, stop=True)
                    pvb = wp.tile([128, 192], dt.bfloat16, tag="tabwv")
                    nc.vector.tensor_copy(pvb[:], pv[:, 0:192])
                    nc.sync.dma_start(tab_vw[rsl, 0:192], pvb[:])

                for w in range(NW if ABL < 3 else 0):
                    if ABL < 23:
                        aggS = wp.tile([128, 450], dt.float32, tag="aggS")
                    for h in range(4):
                        cbase = w * TW + h * HT
                        csl = slice(cbase * 8, (cbase + HT) * 8)
                        pa = gp.tile([128, HT, S], dt.bfloat16, tag="pa")
                        nc.gpsimd.dma_gather(pa[:], tab_src[:], eidx_src[:, csl],
                                             num_idxs=CH, num_idxs_reg=CH,
                                             elem_size=S)
                        pbg = gp.tile([128, HT, S], dt.bfloat16, tag="pb")
                        nc.gpsimd.dma_gather(pbg[:], tab_dst[:], eidx_dst[:, csl],
                                             num_idxs=CH, num_idxs_reg=CH,
                                             elem_size=S)
                        vwg = gp.tile([128, HT, S], dt.bfloat16, tag="vw")
                        nc.gpsimd.dma_gather(vwg[:], tab_vw[:], eidx_src[:, csl],
                                             num_idxs=CH, num_idxs_reg=CH,
                                             elem_size=S)
                        for tt in range(HT if ABL < 25 else 0):
                            gt = cbase + tt
                            rbft = sp.tile([32, 128], dt.bfloat16, tag="rbft")
                            nc.sync.dma_start(rbft[:], i_rbfT[:, gt, :])
                            esc = sp.tile([128, 8], dt.float32, tag="esc")
                            nc.sync.dma_start(esc[:], i_escal[gt])
                            pm = pmm.tile([128, S], dt.float32, tag="mm")
                            nc.tensor.matmul(pm[:], rbft[:], Wer[:, l, :],
                                             start=True, stop=True)
                            mm = wp.tile([128, S], dt.float32, tag="mm")
                            nc.vector.tensor_add(mm[:], pa[:, tt, :], pbg[:, tt, :])
                            nc.vector.tensor_add(mm[:], mm[:], pm[:])
                            msil = wp.tile([128, S], dt.float32, tag="msil")
                            nc.scalar.activation(msil[:], mm[:], AF.Silu)
                            pay = wp.tile([128, 448], dt.bfloat16, tag="pay")
                            nc.vector.tensor_scalar_mul(pay[:, 0:S], msil[:],
                                                        esc[:, 3:4])
                            if ABL >= 24:
                                continue
                            mT = wp.tile([128, 2, 128], dt.bfloat16, tag="mT")
                            for kc in range(2):
                                px = pt.tile([128, 128], dt.bfloat16, tag="tr")
                                nc.tensor.transpose(
                                    px[:], pay[:, kc * 128:(kc + 1) * 128], ident[:])
                                nc.vector.tensor_copy(mT[:, kc, :], px[:])
                            pgt = pg.tile([128, V], dt.float32, tag="gate")
                            for kc in range(2):
                                nc.tensor.matmul(pgt[:], mT[:, kc, :],
                                                 Wv[:, l, kc, :],
                                                 start=(kc == 0), stop=(kc == 1))
                            for dd in range(3):
                                nc.vector.tensor_scalar_mul(
                                    pay[:, S + V * dd:S + V * (dd + 1)],
                                    pgt[:], esc[:, dd:dd + 1])
                            nc.vector.tensor_add(pay[:, S:S + 192],
                                                 pay[:, S:S + 192],
                                                 vwg[:, tt, 0:192])
                            if ABL >= 23:
                                continue
                            oh = wp.tile([128, 128], dt.bfloat16, tag="oh")
                            nc.vector.tensor_scalar(oh[:], iota[:], esc[:, 5:6],
                                                    None, op0=OP.is_equal)
                            psA = pagg.tile([128, 450], dt.float32)
                            nc.tensor.matmul(psA[:, 0:448], oh[:], pay[:],
                                             start=True, stop=True)
                            nc.tensor.matmul(psA[:, 448:450], oh[:], ones2[:],
                                             start=True, stop=True)
                            if h == 0 and tt == 0:
                                nc.vector.tensor_copy(aggS[:], psA[:])
                            else:
                                nc.vector.tensor_add(aggS[:], aggS[:], psA[:])
                    if ABL >= 23:
                        continue
                    wsl = slice(w * 128, (w + 1) * 128)  # epilogue runs for ABL<=22
                    cntc = sp.tile([128, 1], dt.float32, tag="cnt")
                    nc.vector.tensor_scalar_max(cntc[:], aggS[:, 448:449], 1.0)
                    inv = sp.tile([128, 1], dt.float32, tag="inv")
                    nc.vector.reciprocal(inv[:], cntc[:])
                    aggm = wp.tile([128, S], dt.bfloat16, tag="aggm")
                    nc.vector.tensor_scalar_mul(aggm[:], aggS[:, 0:S], inv[:])
                    aggv = wp.tile([128, 192], dt.float32, tag="aggv")
                    nc.vector.tensor_scalar_mul(aggv[:], aggS[:, S:S + 448 - S], inv[:])
                    aT = wp.tile([128, 2, 128], dt.bfloat16, tag="mT")
                    for kc in range(2):
                        px = pt.tile([128, 128], dt.bfloat16, tag="tr")
                        nc.tensor.transpose(px[:], aggm[:, kc * 128:(kc + 1) * 128],
                                            ident[:])
                        nc.vector.tensor_copy(aT[:, kc, :], px[:])
                    for fc in range(2):
                        ph = pt.tile([128, 128], dt.float32, tag="tr")
                        for kc in range(2):
                            nc.tensor.matmul(ph[:],
                                             Wsp[:, l, kc, fc * 128:(fc + 1) * 128],
                                             aT[:, kc, :], start=(kc == 0),
                                             stop=(kc == 1))
                        dl = wp.tile([128, 128], dt.float32, tag="dl")
                        nc.scalar.activation(dl[:], ph[:], AF.Silu,
                                             bias=bsT[:, l, fc:fc + 1])
                        nc.vector.tensor_add(sT_f[:, fc, wsl], sT_f[:, fc, wsl],
                                             dl[:])
                    for dd in range(3):
                        px = pt.tile([128, 128], dt.float32, tag="tr")
                        nc.tensor.transpose(px[0:V, :], aggv[:, dd * V:(dd + 1) * V],
                                            identf[:])
                        nc.vector.tensor_add(vT_f[:, dd, wsl], vT_f[:, dd, wsl],
                                             px[0:V, :])

                # layernorm on own nodes
                pmu = pmm.tile([1, NS], dt.float32, tag="mm")
                for fc in range(2):
                    nc.tensor.matmul(pmu[:], ones_cf[:], sT_f[:, fc, :],
                                     start=(fc == 0), stop=(fc == 1))
                pms = pmm.tile([1, NS], dt.float32, tag="mm")
                for fc in range(2):
                    sq2 = wp.tile([128, NS], dt.float32, tag="z")
                    nc.vector.tensor_tensor(sq2[:], sT_f[:, fc, :], sT_f[:, fc, :],
                                            op=OP.mult)
                    nc.tensor.matmul(pms[:], ones_cf[:], sq2[:],
                                     start=(fc == 0), stop=(fc == 1))
                mur = sp.tile([1, NS], dt.float32, tag="lnr")
                nc.vector.tensor_scalar_mul(mur[:], pmu[:], 1.0 / S)
                varr = sp.tile([1, NS], dt.float32, tag="lnr")
                nc.vector.tensor_tensor(varr[:], mur[:], mur[:], op=OP.mult)
                pMU = pmm.tile([128, NS], dt.float32, tag="mm")
                nc.tensor.matmul(pMU[:], ones_rf[:], mur[:], start=True, stop=True)
                msr = sp.tile([1, NS], dt.float32, tag="lnr")
                nc.vector.tensor_scalar_mul(msr[:], pms[:], 1.0 / S)
                nc.vector.tensor_sub(varr[:], msr[:], varr[:])
                nc.vector.tensor_scalar_add(varr[:], varr[:], 1e-5)
                sd = sp.tile([1, NS], dt.float32, tag="lnr")
                nc.scalar.activation(sd[:], varr[:], AF.Sqrt)
                rs = sp.tile([1, NS], dt.float32, tag="lnr")
                nc.vector.reciprocal(rs[:], sd[:])
                pRS = pmm.tile([128, NS], dt.float32, tag="mm")
                nc.tensor.matmul(pRS[:], ones_rf[:], rs[:], start=True, stop=True)
                for fc in range(2):
                    z = wp.tile([128, NS], dt.float32, tag="z")
                    nc.vector.tensor_sub(z[:], sT_f[:, fc, :], pMU[:])
                    nc.vector.tensor_tensor(z[:], z[:], pRS[:], op=OP.mult)
                    nc.vector.tensor_scalar(sT_f[:, fc, :], z[:],
                                            lgT[:, l, fc:fc + 1],
                                            lbT[:, l, fc:fc + 1],
                                            op0=OP.mult, op1=OP.add)
                allgather_state()

            # ---------------- heads ----------------
            for fc in range(2 if ABL < 2 else 0):
                for cb in range(N // 512):
                    sl = slice(cb * 512, (cb + 1) * 512)
                    ph = pmm.tile([128, 512], dt.float32, tag="mm")
                    for kc in range(2):
                        nc.tensor.matmul(ph[:], shW[:, kc, fc * 128:(fc + 1) * 128],
                                         sT_bf[:, kc, sl], start=(kc == 0),
                                         stop=(kc == 1))
                    s2f = wp.tile([128, 512], dt.float32, tag="z")
                    nc.scalar.activation(s2f[:], ph[:], AF.Silu,
                                         bias=shbT[:, fc:fc + 1])
                    nc.vector.tensor_copy(s2T[:, fc, sl], s2f[:])
            # p2 table + s2 row table (reuse tab_src)
            for nt in range(N // 128 if ABL < 2 else 0):
                rsl = slice(nt * 128, (nt + 1) * 128)
                ps = pmm.tile([128, S], dt.float32, tag="mm")
                for kc in range(2):
                    nc.tensor.matmul(ps[:], s2T[:, kc, rsl], b0W[:, kc, :],
                                     start=(kc == 0), stop=(kc == 1))
                pb_ = wp.tile([128, S], dt.bfloat16, tag="tabw")
                nc.vector.tensor_copy(pb_[:], ps[:])
                nc.sync.dma_start(tab_p2[rsl, :], pb_[:])
                s2r = wp.tile([128, S], dt.bfloat16, tag="tabw")
                for kc in range(2):
                    px = pt.tile([128, 128], dt.bfloat16, tag="tr")
                    nc.tensor.transpose(px[:], s2T[:, kc, rsl], ident[:])
                    nc.vector.tensor_copy(s2r[:, kc * 128:(kc + 1) * 128], px[:])
                nc.sync.dma_start(tab_src[rsl, :], s2r[:])

            # coords: cv = v @ co_W for all nodes, then center per graph
            for cb in range(N // 128 if ABL < 2 else 0):
                rsl = slice(cb * 128, (cb + 1) * 128)
                pcv = pg.tile([128, V], dt.float32, tag="gate")
                for dd in range(3):
                    nc.tensor.matmul(pcv[:, dd:dd + 1], vT_bf[:, dd, rsl], coW[:],
                                     start=True, stop=True)
                nc.vector.tensor_copy(cvr[:, cb, :], pcv[:, 0:3])
                nc.vector.tensor_copy(cvb[:, cb, :], pcv[:, 0:3])
            HEADS = ABL < 2
            B1 = gp.tile([128, 32, 128], dt.bfloat16, tag="pa")
            nc.sync.dma_start(B1[:], i_B1.ap().rearrange("c p g -> p c g"))
            B1T = gp.tile([128, 32, 128], dt.bfloat16, tag="pb")
            nc.sync.dma_start(B1T[:], i_B1T.ap().rearrange("c p g -> p c g"))
            pgm = pg.tile([128, V], dt.float32, tag="gate")
            for cb in range(N // 128 if HEADS else 0):
                nc.tensor.matmul(pgm[:, 0:3], B1[:, cb, :], cvb[:, cb, :],
                                 start=(cb == 0), stop=(cb == N // 128 - 1))
            if not HEADS:
                nc.tensor.matmul(pgm[:, 0:3], B1[:, 0, :], cvb[:, 0, :],
                                 start=True, stop=True)
            gm = sp.tile([128, 3], dt.float32, tag="gm")
            nc.vector.tensor_scalar_mul(gm[:], pgm[:, 0:3], invc[:])
            gmb = sp.tile([128, 3], dt.bfloat16, tag="gmb")
            nc.vector.tensor_copy(gmb[:], gm[:])
            ctf = wp.tile([128, V], dt.float32, tag="ctf")
            nc.vector.memset(ctf[:], 0.0)
            for cb in range(N // 128 if HEADS else 0):
                rsl = slice(cb * 128, (cb + 1) * 128)
                pe = pg.tile([128, V], dt.float32, tag="gate")
                nc.tensor.matmul(pe[:, 0:3], B1T[:, cb, :], gmb[:],
                                 start=True, stop=True)
                pct = sp.tile([128, 3], dt.float32, tag="pct")
                nc.sync.dma_start(pct[:], i_pc[cb])
                crw = sp.tile([128, 3], dt.float32, tag="crw")
                nc.vector.tensor_sub(crw[:], cvr[:, cb, :], pe[:, 0:3])
                nc.vector.tensor_add(crw[:], crw[:], pct[:])
                ctf2 = wp.tile([128, V], dt.float32, tag="ctf")
                nc.vector.tensor_copy(ctf2[:], ctf[:])
                nc.vector.tensor_copy(ctf2[:, 0:3], crw[:])
                nc.sync.dma_start(tab_cof[rsl, :], ctf2[:])

            # own coords + atoms via gathers
            if not HEADS:
                nc.vector.memset(cvb[:], 0.0)
            ocg = gp.tile([128, NS // 128, V], dt.float32, tag="ci")
            nc.gpsimd.dma_gather(ocg[:], tab_cof[:], own_idx[:], num_idxs=NS,
                                 num_idxs_reg=NS, elem_size=V)
            for cb in range(NS // 128):
                nc.sync.dma_start(
                    o_coords.ap().rearrange("(a p) d -> p a d", p=128)[:, cb, :],
                    ocg[:, cb, 0:3])
            os2 = gp.tile([128, NS // 128, S], dt.bfloat16, tag="pa")
            nc.gpsimd.dma_gather(os2[:], tab_src[:], own_idx[:], num_idxs=NS,
                                 num_idxs_reg=NS, elem_size=S)
            for cb in range(NS // 128):
                s2To = wp.tile([128, 2, 128], dt.bfloat16, tag="mT")
                for kc in range(2):
                    px = pt.tile([128, 128], dt.bfloat16, tag="tr")
                    nc.tensor.transpose(px[:], os2[:, cb, kc * 128:(kc + 1) * 128],
                                        ident[:])
                    nc.vector.tensor_copy(s2To[:, kc, :], px[:])
                pat = pg.tile([128, V], dt.float32, tag="gate")
                for kc in range(2):
                    nc.tensor.matmul(pat[:, 0:KA], s2To[:, kc, :], at2W[:, kc, :],
                                     start=(kc == 0), stop=(kc == 1))
                arow = sp.tile([128, KA], dt.float32, tag="arow")
                nc.vector.tensor_add(arow[:], pat[:, 0:KA], at2bB[:])
                nc.sync.dma_start(
                    o_atoms.ap().rearrange("(a p) d -> p a d", p=128)[:, cb, :],
                    arow[:])

            # bonds head
            nbc = (NBT + BCH - 1) // BCH if ABL < 1 else 0
            for c in range(nbc):
                t0 = c * BCH
                ct = min(BCH, NBT - t0)
                cn = ct * 128
                csl = slice(t0 * 8, t0 * 8 + cn // 16)
                fi = gp.tile([128, BCH, S], dt.bfloat16, tag="pa")
                nc.gpsimd.dma_gather(fi[:, 0:ct, :], tab_p2[:], bond_i[:, csl],
                                     num_idxs=cn, num_idxs_reg=cn, elem_size=S)
                fj = gp.tile([128, BCH, S], dt.bfloat16, tag="pb")
                nc.gpsimd.dma_gather(fj[:, 0:ct, :], tab_p2[:], bond_j[:, csl],
                                     num_idxs=cn, num_idxs_reg=cn, elem_size=S)
                ci = gp.tile([128, BCH, V], dt.float32, tag="ci")
                nc.gpsimd.dma_gather(ci[:, 0:ct, :], tab_cof[:], bond_i[:, csl],
                                     num_idxs=cn, num_idxs_reg=cn, elem_size=V)
                cj = gp.tile([128, BCH, V], dt.float32, tag="cj")
                nc.gpsimd.dma_gather(cj[:, 0:ct, :], tab_cof[:], bond_j[:, csl],
                                     num_idxs=cn, num_idxs_reg=cn, elem_size=V)
                for tt in range(ct):
                    gt = t0 + tt
                    dsub = sp.tile([128, 3], dt.float32, tag="dsub")
                    nc.vector.tensor_sub(dsub[:], ci[:, tt, 0:3], cj[:, tt, 0:3])
                    dq = sp.tile([128, 3], dt.float32, tag="dq")
                    nc.vector.tensor_tensor(dq[:], dsub[:], dsub[:], op=OP.mult)
                    ds = sp.tile([128, 1], dt.float32, tag="ds")
                    nc.vector.tensor_reduce(ds[:], dq[:], axis=mybir.AxisListType.X,
                                            op=OP.add)
                    nc.vector.tensor_scalar_max(ds[:], ds[:], 1e-12)
                    dcol = sp.tile([128, 1], dt.float32, tag="dcol")
                    nc.scalar.activation(dcol[:], ds[:], AF.Sqrt)
                    e1 = wp.tile([128, S], dt.float32, tag="mm")
                    nc.vector.tensor_scalar_mul(e1[:], wdB[:], dcol[:])
                    nc.vector.tensor_add(e1[:], e1[:], b0bB[:])
                    nc.vector.tensor_add(e1[:], e1[:], fi[:, tt, :])
                    nc.vector.tensor_add(e1[:], e1[:], fj[:, tt, :])
                    es = wp.tile([128, S], dt.float32, tag="msil")
                    nc.scalar.activation(es[:], e1[:], AF.Silu)
                    esb = wp.tile([128, S], dt.bfloat16, tag="pay")
                    nc.vector.tensor_copy(esb[:], es[:])
                    eT = wp.tile([128, 2, 128], dt.bfloat16, tag="mT")
                    for kc in range(2):
                        px = pt.tile([128, 128], dt.bfloat16, tag="tr")
                        nc.tensor.transpose(px[:], esb[:, kc * 128:(kc + 1) * 128],
                                            ident[:])
                        nc.vector.tensor_copy(eT[:, kc, :], px[:])
                    pb5 = pg.tile([128, V], dt.float32, tag="gate")
                    for kc in range(2):
                        nc.tensor.matmul(pb5[:, 0:NB], eT[:, kc, :], b1W[:, kc, :],
                                         start=(kc == 0), stop=(kc == 1))
                    ob = sp.tile([128, NB], dt.float32, tag="ob")
                    nc.vector.tensor_add(ob[:], pb5[:, 0:NB], b1bB[:])
                    nc.sync.dma_start(
                        o_bonds.ap().rearrange("(a p) d -> p a d", p=128)[:, gt, :],
                        ob[:])

    nc.compile()
    return nc


_CACHE = {}


def kernel(x, t, pos, edge_index_local, edge_index_global, batch, params):
    x = np.asarray(x, F32)
    t = np.asarray(t, F32)
    pos = np.asarray(pos, F32)
    eil = np.asarray(edge_index_local).astype(np.int64)
    eig = np.asarray(edge_index_global).astype(np.int64)
    batch = np.asarray(batch).astype(np.int64)
    p = {k: np.asarray(v, F32) for k, v in params.items()}

    # ---- host prep: geometry ----
    cnt = np.bincount(batch, minlength=G).astype(F32)
    cnt1 = np.maximum(cnt, 1.0)
    gsum = np.zeros((G, 3), F32)
    np.add.at(gsum, batch, pos)
    pos_c = pos - (gsum / cnt1[:, None])[batch]
    gsum2 = np.zeros((G, 3), F32)
    np.add.at(gsum2, batch, pos_c)
    pc = pos_c - (gsum2 / cnt1[:, None])[batch]

    src = np.concatenate([eil[0], eig[0]])
    dst = np.concatenate([eil[1], eig[1]])
    rvec = pos_c[dst] - pos_c[src]
    d = np.sqrt(np.maximum((rvec * rvec).sum(-1), 1e-6))
    rn = rvec / d[:, None]
    dL = d[:E_L]
    env = np.concatenate([
        (0.5 * (np.cos(np.pi * dL / CUTOFF) + 1.0) * (dL < CUTOFF)).astype(F32),
        np.ones(E_G, F32)])
    centers = np.linspace(0.0, CUTOFF, R).astype(F32)
    rbf = np.exp(-(R / CUTOFF) * (d[:, None] - centers) ** 2).astype(F32)

    # ---- sort edges by dst, shard by dst range, window by 128 ----
    order = np.argsort(dst, kind="stable")
    dst_s = dst[order]
    starts = np.searchsorted(dst_s, np.arange(0, N + 1, 128))
    wcnt = np.diff(starts)          # edges per (core,window) flat [32]
    TW = int(np.ceil(wcnt.max() / 128.0))
    TW = ((TW + 3) // 4) * 4
    NT = NW * TW
    EP = NT * 128

    key = TW
    if key not in _CACHE:
        _CACHE[key] = _build(TW)
    nc = _CACHE[key]

    def bc(a):
        return np.ascontiguousarray(a).astype(BF)

    # per-core padded edge arrays
    in_maps = []
    eye = np.eye(128, dtype=F32)
    iota = np.tile(np.arange(128, dtype=F32)[None, :], (128, 1))
    common = dict(
        Wes=bc(p["We"][:, :S].reshape(L, 2, 128, S)),
        Wed=bc(p["We"][:, S:2 * S].reshape(L, 2, 128, S)),
        Wer=bc(p["We"][:, 2 * S:]),
        be=bc(p["be"].reshape(L, 1, S)),
        Ws=bc(p["Ws"].reshape(L, 2, 128, S)),
        bsT=np.ascontiguousarray(
            p["bs"].reshape(L, 2, 128).transpose(0, 2, 1)).astype(F32),
        Wv=bc(p["Wv"].reshape(L, 2, 128, V)),
        Wvv=bc(p["Wvv"]),
        lgT=np.ascontiguousarray(
            p["ln_g"].reshape(L, 2, 128).transpose(0, 2, 1)).astype(F32),
        lbT=np.ascontiguousarray(
            p["ln_b"].reshape(L, 2, 128).transpose(0, 2, 1)).astype(F32),
        atW=bc(p["atom_W"]),
        atW2=bc(p["at_W"].reshape(2, 128, S)),
        atbT=np.ascontiguousarray(p["at_b"].reshape(2, 128).T).astype(F32),
        shW=bc(p["sh_W"].reshape(2, 128, S)),
        shbT=np.ascontiguousarray(p["sh_b"].reshape(2, 128).T).astype(F32),
        b0W=bc(p["b0_W"][:S].reshape(2, 128, S)),
        wdB=np.tile(p["b0_W"][S][None, :], (128, 1)).astype(F32),
        b0bB=np.tile(p["b0_b"][None, :], (128, 1)).astype(F32),
        b1W=bc(p["b1_W"].reshape(2, 128, NB)),
        b1bB=np.tile(p["b1_b"][None, :], (128, 1)).astype(F32),
        coW=bc(p["co_W"]),
        at2W=bc(p["at2_W"].reshape(2, 128, KA)),
        at2bB=np.tile(p["at2_b"][None, :], (128, 1)).astype(F32),
        invc=(1.0 / cnt1).reshape(128, 1).astype(F32),
        ident=bc(eye),
        identf=eye.astype(F32),
        iota=iota.astype(F32),
        ones_r=bc(np.ones((1, 128), F32)),
        ones_rf=np.ones((1, 128), F32),
        ones_cf=np.ones((128, 1), F32),
        ones2=np.ones((128, 2), BF),
        pc_all=np.ascontiguousarray(pc.reshape(N // 128, 128, 3)).astype(F32),
    )
    B1 = np.zeros((N, G), F32)
    B1[np.arange(N), batch] = 1.0
    common["B1"] = bc(B1.reshape(32, 128, 128))
    common["B1T"] = bc(
        np.ascontiguousarray(B1.T).reshape(128, 32, 128).transpose(1, 0, 2))

    ta = (t @ p["time_W"] + p["time_b"]).astype(F32)
    ta2 = ta[batch] + p["atom_b"][None, :]

    for c in range(CORES):
        src_p = np.full(EP, N, np.int64)
        dst_p = np.full(EP, N, np.int64)
        esc = np.zeros((EP, 8), F32)
        esc[:, 5] = -1.0
        rbf_p = np.zeros((EP, R), F32)
        for w in range(NW):
            gwi = c * NW + w
            s0, s1 = starts[gwi], starts[gwi + 1]
            idxs = order[s0:s1]
            n = s1 - s0
            base = w * TW * 128
            src_p[base:base + n] = src[idxs]
            dst_p[base:base + n] = dst[idxs]
            esc[base:base + n, 0:3] = rn[idxs]
            esc[base:base + n, 3] = env[idxs]
            esc[base:base + n, 4] = 1.0
            esc[base:base + n, 5] = (dst[idxs] - (c * NS + w * 128)).astype(F32)
            rbf_p[base:base + n] = rbf[idxs]
        own = np.arange(c * NS, (c + 1) * NS)
        m = dict(common)
        m.update(
            eidx_src=_wrap_idx(src_p),
            eidx_dst=_wrap_idx(dst_p),
            own_idx=_wrap_idx(own),
            bond_i=_wrap_idx(eig[1][c * EG_SH:(c + 1) * EG_SH]),
            bond_j=_wrap_idx(eig[0][c * EG_SH:(c + 1) * EG_SH]),
            rbfT=bc(rbf_p.reshape(NT, 128, R).transpose(2, 0, 1)),
            escal=np.ascontiguousarray(esc.reshape(NT, 128, 8)),
            xT_own=bc(x[own].T),
            ta2T_own=np.ascontiguousarray(
                ta2[own].T.reshape(2, 128, NS).transpose(1, 0, 2)).astype(F32),
        )
        in_maps.append(m)

    res = run_bass_kernel_spmd(nc, in_maps, core_ids=list(range(CORES)),
                               trace=False)
    coords = np.concatenate([r["coords_sh"] for r in res.results], 0)
    atoms = np.concatenate([r["atoms_sh"] for r in res.results], 0)
    bonds = np.concatenate([r["bonds_sh"] for r in res.results], 0)
    return coords, atoms, bonds
